# revision 39
# baseline (speedup 1.0000x reference)
"""Bass/Trainium2 kernel for masked (padding) multi-head self-attention.

Problem: B=2, T=2048, C=1024, H=16 heads of DH=64.
  q/k/v = x @ W* + b*  ->  att = softmax(mask(q k^T / 8))  ->  y = att @ v

Sharding over 8 NeuronCores: core = (batch b, head-group hg) with
b = core // 4, hg = core % 4; each core computes 4 heads for one batch
element (its [T, 256] slice of q/k/v from the Wq/Wk/Wv column slice).

Host-side preprocessing (inside kernel()):
  - Tokens with mask==0 contribute nothing, so we gather only the valid
    tokens per batch; k-tiles pad to a multiple of 128 (nkt tiles), the
    query dim is trimmed to QV = nch*cw ~ max_valid columns.
  - All inputs are packed partition-major on the host and cast to bf16,
    so every input lands in one (or a few) large full-rate DMAs.
  - bf16 matmuls (fp32 PSUM accumulation):
      qT[d,q] = sum_c Wq[c,d] xT[c,q]     (lhsT=Wq tile,  rhs=xT)
      kT[d,k] = sum_c Wk[c,d] xT[c,k]
      v[k,d]  = sum_c xT[c,k] Wv[c,d]     (lhsT=xT tile,  rhs=Wv)
      sT[k,q] = sum_d kT[d,k] qT[d,q]     (lhsT=kT slice, rhs=qT)
      e       = bf16(exp(0.125*sT + bias_k))   (bias_k = -1e30 pad rows)
      y[q,de] = sum_k e[k,q] vaug[k,de]   (lhsT=e q-block, rhs=[v | 1])
    The AV product uses e as the stationary operand so the moving size
    is only DH+1=65; column 64 of vaug is a ones column producing the
    softmax denominator for free.
  - Normalization (divide by denominator) and the scatter back to
    [T, C] layout happen on the host during unsharding.

Schedule (instruction-cost-model driven, 60.2us on the timeline sim
vs the 77.4us fp32r baseline; the kernel is PE-bound - the tensor
engine is ~100% busy from the first xt arrival to the last AV chain):
  - 8 long PE warmup matmuls keep the tensor engine's p-state ramp hot
    through the DMA window (cold matmuls cost 2-4x).
  - critical path to the first exp (~13.8us): qT-d0 chunks + kT-d0
    chunk 0 only, pipelined per c-tile against the 8 xt DMAs; their
    evacuations are spread over DVE and ACT (GPSIMD cannot touch PSUM
    on TRN2 - the BIR verifier rejects it).  Dependency tracking is
    tile-granular, so the k chunk is evacuated in ONE op - a split
    would put the late half on the first-score-tile's wait list.
  - the 36-exp ACT stream (~38us busy) then runs dense; everything
    else (kT-d0 rest, d1 projections, v projection, AV chains) is
    split into <=0.6us piece-wise chain items popped from a queue
    between score tiles (kd0+d1 before head-2 scores, v before any AV
    chain; a small per-slot budget keeps the PE one score tile ahead).
  - y [q,65] PSUM chains evac per (head, q-block); heads 0-2 write out
    with one merged DMA each; the last head does a bulk DMA for
    qb 0..n-2 overlapped with its final chain plus a tiny last DMA,
    keeping the post-exp tail to ~8.6us (9 exp-gated chains on 2 PSUM
    slots + the HWDGE/DGE/sem latency of the final transfer).
  Timeline-sim profile: PE 50.4us busy (projections 41k cycles,
  scores 37.3k, AV 21.1k in bf16 at 1 col/cycle + warmup/ramp ~3.5k
  equivalent), ACT exp stream perfectly dense 13.8->51.5us, ~5us of
  unavoidable PE idle (xt DMA pacing at the head, final-exp gating at
  the tail), total 60.2us.  Measured rel err 8.0e-3 (bf16 inputs
  ~6.5e-3 + bf16 e/v ~1.5e-3) vs the 2e-2 gate.  Known-negative
  experiments (all reverted): fp8 DoubleRow projections (6e-2 err),
  ACT y-evacs in the drain (+9us scheduler serialization), paired
  PSUM chains via memset+start=False, sps-pool reuse for tail chains
  (pool boundary = barrier), pre-positioned part-A AV chains for the
  last head (delays the exp stream), larger per-slot pop budgets
  (break exp-stream density).
"""

import math
import sys

sys.path.insert(0, "/opt/trn_rl_repo")

import ml_dtypes
import numpy as np

import concourse.bacc as bacc
import concourse.mybir as mybir
import concourse.tile as tile
from concourse import bass_utils

F32 = mybir.dt.float32
BF16 = mybir.dt.bfloat16
AF = mybir.ActivationFunctionType

B, T, C, H = 2, 2048, 1024, 16
DH = C // H            # 64
HPC = 4                # heads per core
CSL = HPC * DH         # 256, per-core column slice of C
N_CORES = 8

_CACHE: dict = {}

CFG = {
    "dummies": 8,
    "prepop": 0,
    "budgets": [700, 700, 700, 700],
    "spread_evac": True,
    "aux2_first": True,
    "h3_split": True,
    "acta": True,
    "actdrain": False,
    "spsdrain": False,
    "h3_ab": False,
    "pairdrain": False,
}


def _pick_dims(max_valid: int):
    """nkt k-tiles of 128 covering max_valid; query dim trimmed to
    QV = nch*cw with cw <= 512 (PSUM bank) and nch <= 3 (bank budget)."""
    mv = max(max_valid, 128)
    nkt = math.ceil(mv / 128)
    nch = math.ceil(mv / 512)
    if nch > 3:
        raise NotImplementedError(f"too many valid tokens ({max_valid})")
    cw = math.ceil(mv / nch)
    return nkt, nch, cw


def _chunks(total, width):
    out, off = [], 0
    while off < total:
        w = min(width, total - off)
        out.append((off, w))
        off += w
    return out


def _build(nkt: int, nch: int, cw: int, with_bqk: bool):
    tp = nkt * 128          # padded k length
    qv = nch * cw           # trimmed q length
    nqb = math.ceil(qv / 128)
    NCT = C // 128          # 8 contraction tiles over C

    nc = bacc.Bacc("TRN2", target_bir_lowering=False, debug=False,
                   num_devices=N_CORES)

    # all inputs packed partition-major on the host: [128, NCT, ...]
    xt_d = nc.dram_tensor("xt", [128, NCT, tp], BF16, kind="ExternalInput")
    wq0_d = nc.dram_tensor("wq0", [128, NCT, 128], BF16, kind="ExternalInput")
    wk0_d = nc.dram_tensor("wk0", [128, NCT, 128], BF16, kind="ExternalInput")
    # aux1 = wv [., ., 256]; aux2 = wq d1 | wk d1
    aux1_d = nc.dram_tensor("aux1", [128, NCT, 256], BF16,
                            kind="ExternalInput")
    aux2_d = nc.dram_tensor("aux2", [128, NCT, 256], BF16,
                            kind="ExternalInput")
    # misc128: col 0..3 = bqk (bq d0, bq d1, bk d0, bk d1), col 4.. = ebias
    misc128_d = nc.dram_tensor("misc128", [128, 4 + nkt], F32,
                               kind="ExternalInput")
    out_d = nc.dram_tensor("out", [HPC, 128, nqb, DH + 1], F32,
                           kind="ExternalOutput")

    qchunks = _chunks(qv, cw)        # q-projection chunks (=score chunks)
    kchunks = _chunks(tp, 384)       # k-projection chunks

    with tile.TileContext(nc) as tc:
        with tc.tile_pool(name="const", bufs=1) as cp:
            xt_sb = cp.tile([128, NCT, tp], BF16, tag="xt")
            wq0_sb = cp.tile([128, NCT, 128], BF16, tag="wq0")
            wk0_sb = cp.tile([128, NCT, 128], BF16, tag="wk0")
            aux1_sb = cp.tile([128, NCT, 256], BF16, tag="aux1")
            aux2_sb = cp.tile([128, NCT, 256], BF16, tag="aux2")
            misc128_sb = cp.tile([128, 4 + nkt], F32, tag="misc128")
            qt_sb = [cp.tile([128, qv], BF16, tag=f"qt{p}", name=f"qt{p}")
                     for p in range(2)]
            kt_sb = [cp.tile([128, tp], BF16, tag=f"kt{p}", name=f"kt{p}")
                     for p in range(2)]
            v_sb = cp.tile([128, nkt, HPC, DH + 1], BF16, tag="v")
            e_sb = [[cp.tile([128, nch, cw], BF16, tag=f"e{h}_{t}",
                             name=f"e{h}_{t}")
                     for t in range(nkt)] for h in range(HPC)]
            ystage = [cp.tile([128, nqb, DH + 1], F32, tag=f"ys{h}",
                              name=f"ys{h}") for h in range(HPC)]
            bqk_sb = misc128_sb[:, 0:4]
            ebias_sb = misc128_sb[:, 4:4 + nkt]
            wv_sb = aux1_sb
            w1_sb = {0: aux2_sb[:, :, 0:128], 2: aux2_sb[:, :, 128:256]}

            scratch = cp.tile([1, 8], F32, tag="scratch")

            # DMA order = need order: W d0 halves, xt (8 c-tiles), misc
            # (ebias, needed at first exp), then d1 W and wv.
            nc.sync.dma_start(wq0_sb[:], wq0_d.ap()[:])
            nc.sync.dma_start(wk0_sb[:], wk0_d.ap()[:])
            for i in range(NCT):
                nc.sync.dma_start(xt_sb[:, i, :], xt_d.ap()[:, i, :])
            nc.sync.dma_start(misc128_sb[:], misc128_d.ap()[:])
            if CFG["aux2_first"]:
                nc.sync.dma_start(aux2_sb[:], aux2_d.ap()[:])
                nc.sync.dma_start(aux1_sb[:], aux1_d.ap()[:])
            else:
                nc.sync.dma_start(aux1_sb[:], aux1_d.ap()[:])
                nc.sync.dma_start(aux2_sb[:], aux2_d.ap()[:])

            # ones column of vaug (softmax denominator)
            nc.gpsimd.memset(v_sb[:, :, :, DH], 1.0)
            # warm the ACT exp table during the DMA window
            nc.gpsimd.memset(scratch[:], 0.0)
            nc.scalar.activation(scratch[:], scratch[:], AF.Exp)

            def evac_qk(eng, o_sb, ps, off, w, bcol):
                if with_bqk:
                    eng.tensor_scalar_add(o_sb[:, off:off + w], ps[:, 0:w],
                                          bqk_sb[:, bcol:bcol + 1])
                else:
                    eng.tensor_copy(o_sb[:, off:off + w], ps[:, 0:w])

            # ---- phase A: PE warmup + critical d0 projections ----
            with tc.tile_pool(name="pa", bufs=5, space="PSUM") as pa:
                # long warmup matmuls: keep the PE busy (and its p-state
                # ramp running) from ~0.2us until the first xt arrives.
                wsc = cp.tile([128, 512], BF16, tag="wsc")
                nc.gpsimd.memset(wsc[:, 0:16], 0.0)
                wps = pa.tile([16, 512], F32, tag="a", name="wps")
                for _ in range(CFG["dummies"]):
                    nc.tensor.matmul(wps[:], wsc[:, 0:16], wsc[:],
                                     start=True, stop=True)

                # critical groups: k d0 chunk 0 first (its evac gates the
                # first score tiles), then the q d0 chunks.
                crit = [(wk0_sb, kt_sb[0], 2, kchunks[0][0], kchunks[0][1])]
                crit += [(wq0_sb, qt_sb[0], 0, off, w) for off, w in qchunks]
                gps = [pa.tile([128, 512], F32, tag="a", name=f"pqk{gi}")
                       for gi in range(len(crit))]
                for ct in range(NCT):
                    for (w_sb, o_sb, bcol, off, w), ps in zip(crit, gps):
                        nc.tensor.matmul(
                            ps[:, 0:w], w_sb[:, ct, :],
                            xt_sb[:, ct, off:off + w],
                            start=(ct == 0), stop=(ct == NCT - 1),
                        )
                # evacs spread over Pool/DVE/ACT so the first exp isn't
                # gated by one long copy; k chunk 0 split in halves.
                def act_evac(o_sb, ps, off, w, bcol):
                    # GPSIMD cannot touch PSUM on TRN2; ACT is the second
                    # legal evacuator (idle outside the exp stream).
                    nc.scalar.activation(
                        o_sb[:, off:off + w], ps[:, 0:w], AF.Identity,
                        bias=bqk_sb[:, bcol:bcol + 1])

                # single evac for the k chunk: tile-granular dependency
                # tracking makes the first score tile wait on EVERY write
                # to kt/qt, so a split (k0a/k0b) would put the late half
                # on the first-exp critical path.
                (w_sb, o_sb, bcol, off, w), ps = crit[0], gps[0]
                if CFG["acta"]:
                    act_evac(o_sb, ps, off, w, bcol)
                else:
                    evac_qk(nc.vector, o_sb, ps, off, w, bcol)
                for gi, ((w_sb, o_sb, bcol, off, w), ps) in enumerate(
                        zip(crit[1:], gps[1:])):
                    if gi % 3 == 1 and CFG["acta"]:
                        act_evac(o_sb, ps, off, w, bcol)
                    else:
                        evac_qk(nc.vector, o_sb, ps, off, w, bcol)

            # ---- phase B: scores/exp stream + paced work queue ----
            pool_ref = {}
            in_drain = {"x": False}
            with (
                tc.tile_pool(name="sps", bufs=2, space="PSUM") as sps_pool,
                tc.tile_pool(name="ops", bufs=2, space="PSUM") as ops,
            ):
                pool_ref["p"] = ops
                # work items: (cost_cycles, closure).  Chain-units are
                # split in two halves sharing one PSUM tile; the second
                # half finishes the chain and evacuates.
                pre_q: list = []    # kd0 rest + d1 projections
                v_q: list = []      # v projection
                av_q: list = []     # AV chains, appended per head
                av_meta: list = []  # (h, qb) mirror of av_q
                state: dict = {}

                def qk_unit(w_sb, o_sb, bcol, off, w, key):
                    npc = 4 if w >= 512 else 2
                    cpp = NCT // npc      # c-tiles per piece
                    items = []
                    for pi in range(npc):
                        def piece(pi=pi):
                            if pi == 0:
                                ps = ops.tile([128, 512], F32, tag="o",
                                              name="pqk1")
                                state[key] = ps
                            else:
                                ps = state[key]
                            for ct in range(pi * cpp, (pi + 1) * cpp):
                                nc.tensor.matmul(
                                    ps[:, 0:w], w_sb[:, ct, :],
                                    xt_sb[:, ct, off:off + w],
                                    start=(ct == 0), stop=(ct == NCT - 1),
                                )
                            if pi == npc - 1:
                                ps = state.pop(key)
                                evac_qk(nc.vector, o_sb, ps, off, w, bcol)
                        items.append((cpp * w + (250 if pi == npc - 1 else 0),
                                      piece))
                    return items

                # kT d0 remaining chunks
                for off, w in kchunks[1:]:
                    pre_q += qk_unit(wk0_sb, kt_sb[0], 2, off, w,
                                     ("k0", off))
                # d1 projections (q then k)
                for off, w in qchunks:
                    pre_q += qk_unit(w1_sb[0], qt_sb[1], 1, off, w,
                                     ("q1", off))
                for off, w in kchunks:
                    pre_q += qk_unit(w1_sb[2], kt_sb[1], 3, off, w,
                                     ("k1", off))

                def v_unit(t):
                    def half1():
                        ps = ops.tile([128, 512], F32, tag="o", name="pv")
                        state[("v", t)] = ps
                        for ct in range(NCT // 2):
                            nc.tensor.matmul(
                                ps[:, 0:CSL],
                                xt_sb[:, ct, t * 128:(t + 1) * 128],
                                wv_sb[:, ct, :],
                                start=(ct == 0), stop=False,
                            )

                    def half2():
                        ps = state.pop(("v", t))
                        for ct in range(NCT // 2, NCT):
                            nc.tensor.matmul(
                                ps[:, 0:CSL],
                                xt_sb[:, ct, t * 128:(t + 1) * 128],
                                wv_sb[:, ct, :],
                                start=False, stop=(ct == NCT - 1),
                            )
                        nc.vector.tensor_copy(
                            v_sb[:, t, :, 0:DH],
                            ps[:, 0:CSL].rearrange("p (h d) -> p h d", h=HPC))

                    hc = (NCT // 2) * CSL
                    return [(hc, half1), (hc + 250, half2)]

                for t in range(nkt):
                    v_q += v_unit(t)

                def av_item(h, qb, tlo=0, thi=None):
                    thi_ = nkt if thi is None else thi

                    def run():
                        wq_b = min(128, qv - qb * 128)
                        if in_drain["x"] and qb % 2 and CFG["spsdrain"]:
                            # score-PSUM banks are dead in the drain: use
                            # them as extra chain slots to double pipelining
                            yps = sps_pool.tile([128, DH + 1], F32, tag="s",
                                                name="yps2")
                        else:
                            yps = pool_ref["p"].tile([128, DH + 1], F32,
                                                     tag="o", name="yps")
                        for t in range(tlo, thi_):
                            e_ap = e_sb[h][t][:].rearrange("p a b -> p (a b)")
                            nc.tensor.matmul(
                                yps[0:wq_b, :],
                                e_ap[:, qb * 128:qb * 128 + wq_b],
                                v_sb[:, t, h, :],
                                start=(t == tlo), stop=(t == thi_ - 1),
                            )
                        if tlo > 0:
                            # part-B of a split chain: accumulate the last
                            # k-tile's contribution into the staged partial
                            nc.vector.tensor_add(
                                ystage[h][:, qb, :][0:wq_b],
                                ystage[h][:, qb, :][0:wq_b],
                                yps[0:wq_b, :])
                            _dma_out(h, qb, wq_b)
                            return
                        if in_drain["x"] and qb % 2 and CFG["actdrain"]:
                            nc.scalar.activation(
                                ystage[h][:, qb, :][0:wq_b],
                                yps[0:wq_b, :], AF.Identity, bias=0.0)
                        else:
                            nc.vector.tensor_copy(ystage[h][:, qb, :][0:wq_b],
                                                  yps[0:wq_b, :])
                        if thi is None:
                            _dma_out(h, qb, wq_b)

                    return ((thi_ - tlo) * (DH + 1) + 330, run)

                def _dma_out(h, qb, wq_b):
                    if h == HPC - 1 and CFG["h3_split"]:
                        # last head: bulk DMA once qb 0..nqb-2 are done,
                        # then a tiny one for the final block, so the
                        # HWDGE pass overlaps the last AV chain.
                        if qb == nqb - 2:
                            nc.sync.dma_start(
                                out_d.ap()[h, :, 0:nqb - 1, :],
                                ystage[h][:, 0:nqb - 1, :])
                        elif qb == nqb - 1:
                            nc.sync.dma_start(
                                out_d.ap()[h, :, qb, :][0:wq_b],
                                ystage[h][:, qb, :][0:wq_b])
                    elif qb == nqb - 1:
                        nc.sync.dma_start(out_d.ap()[h], ystage[h][:])

                def pop_work(budget):
                    acc = 0
                    while acc < budget:
                        if pre_q:
                            c, fn = pre_q.pop(0)
                        elif v_q:
                            c, fn = v_q.pop(0)
                        elif av_q:
                            c, fn = av_q.pop(0)
                            av_meta.pop(0)
                        else:
                            return
                        fn()
                        acc += c

                def scores(h, t):
                    pd, po = h // 2, (h % 2) * 64
                    ps = sps_pool.tile([128, nch, 512], F32, tag="s",
                                       name="sps")
                    for j, (off, w) in enumerate(qchunks):
                        nc.tensor.matmul(
                            ps[:, j, 0:w],
                            kt_sb[pd][po:po + 64, t * 128:(t + 1) * 128],
                            qt_sb[pd][po:po + 64, off:off + w],
                            start=True, stop=True,
                        )
                    nc.scalar.activation(
                        e_sb[h][t][:], ps[:, :, 0:cw], AF.Exp,
                        bias=ebias_sb[:, t:t + 1], scale=0.125,
                    )

                # bridge the evac-wait before the first score tile with
                # queue work so the PE (and its p-state ramp) stays hot
                pop_work(CFG["prepop"])
                budgets = CFG["budgets"]
                for h in range(HPC):
                    last_ab = h == HPC - 1 and CFG["h3_ab"] and nkt > 1
                    for t in range(nkt - 1 if last_ab else nkt):
                        scores(h, t)
                        pop_work(budgets[h])
                    if last_ab:
                        # final score tile first (keeps the ACT stream
                        # dense), then pre-position the last head's AV
                        # chains: part-A (k-tiles 0..nkt-2) runs under
                        # the exp-stream tail, so after the final exp
                        # only a tiny matmul + add per q-block remains.
                        scores(h, nkt - 1)
                        for qb in range(nqb):
                            av_item(h, qb, 0, nkt - 1)[1]()
                        for qb in range(nqb):
                            av_q.append(av_item(h, qb, nkt - 1, nkt))
                            av_meta.append((h, qb))
                    else:
                        for qb in range(nqb):
                            av_q.append(av_item(h, qb))
                            av_meta.append((h, qb))
                    if h == 1:
                        # d-tile-1 qT/kT must exist before head-2 scores
                        while pre_q:
                            c, fn = pre_q.pop(0)
                            fn()
                def av_pair(itemA, itemB):
                    # two drain chains share one PSUM bank: a DVE memset
                    # zeroes it, both chains accumulate (start=False), so
                    # four chains are in flight across the two ops slots.
                    (hA, qbA), (hB, qbB) = itemA, itemB
                    tile2 = ops.tile([128, 2, DH + 1], F32, tag="o",
                                     name="ypp")
                    nc.vector.memset(tile2[:], 0.0)
                    for j, (h, qb) in enumerate(((hA, qbA), (hB, qbB))):
                        wq_b = min(128, qv - qb * 128)
                        for t in range(nkt):
                            e_ap = e_sb[h][t][:].rearrange("p a b -> p (a b)")
                            nc.tensor.matmul(
                                tile2[0:wq_b, j, :],
                                e_ap[:, qb * 128:qb * 128 + wq_b],
                                v_sb[:, t, h, :],
                                start=False, stop=(t == nkt - 1),
                                skip_group_check=True,
                            )
                        nc.vector.tensor_copy(ystage[h][:, qb, :][0:wq_b],
                                              tile2[0:wq_b, j, :])
                        _dma_out(h, qb, wq_b)

                # drain the remainder (AV of later heads + stragglers)
                in_drain["x"] = True
                while pre_q or v_q:
                    pop_work(1 << 30)
                if CFG["pairdrain"]:
                    while len(av_meta) >= 2:
                        a = av_meta.pop(0)
                        b = av_meta.pop(0)
                        av_q.pop(0)
                        av_q.pop(0)
                        av_pair(a, b)
                while av_q:
                    av_meta and av_meta.pop(0)
                    av_q.pop(0)[1]()

    nc.compile()
    return nc


def _get_nc(nkt, nch, cw, with_bqk):
    key = (nkt, nch, cw, with_bqk)
    if key not in _CACHE:
        _CACHE[key] = _build(nkt, nch, cw, with_bqk)
    return _CACHE[key]


def _pack_pm(a):
    """[C, n] -> partition-major [128, NCT, n] (c = i*128 + p)."""
    n = a.shape[1]
    return np.ascontiguousarray(
        a.reshape(C // 128, 128, n).transpose(1, 0, 2))


def kernel(x, Wq, bq, Wk, bk, Wv, bv, mask):
    x = np.asarray(x, dtype=np.float32)
    Wq = np.asarray(Wq, dtype=np.float32)
    bq = np.asarray(bq, dtype=np.float32)
    Wk = np.asarray(Wk, dtype=np.float32)
    bk = np.asarray(bk, dtype=np.float32)
    Wv = np.asarray(Wv, dtype=np.float32)
    bv = np.asarray(bv, dtype=np.float32)
    mask = np.asarray(mask)

    idxs = [np.nonzero(mask[b] != 0)[0] for b in range(B)]
    tvs = [len(ix) for ix in idxs]
    nkt, nch, cw = _pick_dims(max(max(tvs), 1))
    tp, qv = nkt * 128, nch * cw
    nqb = math.ceil(qv / 128)
    with_bqk = bool(np.any(bq) or np.any(bk))
    nc = _get_nc(nkt, nch, cw, with_bqk)

    bf = ml_dtypes.bfloat16

    xts, ebs = [], []
    for b in range(B):
        xt = np.zeros((C, tp), np.float32)
        if tvs[b]:
            xt[:, :tvs[b]] = x[b][idxs[b]].T
        xts.append(_pack_pm(xt).astype(bf))
        eb = np.full(tp, -1e30, np.float32)
        eb[:tvs[b]] = 0.0
        ebs.append(eb.reshape(nkt, 128).T.copy())

    in_maps = []
    for core in range(N_CORES):
        b, hg = core // HPC, core % HPC
        cs = hg * CSL
        misc128 = np.concatenate([
            np.stack([bq[cs:cs + 128], bq[cs + 128:cs + 256],
                      bk[cs:cs + 128], bk[cs + 128:cs + 256]], axis=1),
            ebs[b],
        ], axis=1)
        aux2 = np.concatenate([Wq[:, cs + 128:cs + 256],
                               Wk[:, cs + 128:cs + 256]], axis=1)
        in_maps.append({
            "xt": xts[b],
            "wq0": _pack_pm(Wq[:, cs:cs + 128]).astype(bf),
            "wk0": _pack_pm(Wk[:, cs:cs + 128]).astype(bf),
            "aux1": _pack_pm(Wv[:, cs:cs + CSL]).astype(bf),
            "aux2": _pack_pm(aux2).astype(bf),
            "misc128": np.ascontiguousarray(misc128),
        })

    try:
        res = bass_utils.run_bass_kernel_spmd(
            nc, in_maps, core_ids=list(range(N_CORES)), trace=False)
    except Exception:
        # transient axon-worker/NRT failures recover on retry
        res = bass_utils.run_bass_kernel_spmd(
            nc, in_maps, core_ids=list(range(N_CORES)), trace=False)

    y = np.zeros((B, T, C), np.float32)
    for core in range(N_CORES):
        b, hg = core // HPC, core % HPC
        out = res.results[core]["out"]       # [HPC, 128, nqb, DH+1]
        ix, tv = idxs[b], tvs[b]
        if not tv:
            continue
        for h in range(HPC):
            o = out[h].transpose(1, 0, 2).reshape(nqb * 128, DH + 1)
            numer = o[:tv, :DH]
            denom = o[:tv, DH:DH + 1]
            col = hg * CSL + h * DH
            y[b, ix, col:col + DH] = numer / denom
            if np.any(bv[col:col + DH]):
                y[b, ix, col:col + DH] += bv[col:col + DH]
    return y
